# revision 1
# baseline (speedup 1.0000x reference)
"""Trainium2 Bass kernel for nn_MistralMoLoraLayer (MoE-routed LoRA FFN).

Strategy: data-parallel over tokens (8 cores x 256 tokens), base FFN weights
replicated, all-expert LoRA replicated. The per-(batch,slot) softmax over the
sequence axis needs global denominators -> tiny [2,8] AllReduce.

Per-core math (all tiles [h/er/d partitions, tokens free]):
  router: logits = x @ gate_w.T; top-2 (value,index) per token; exp; AR of
          per-batch-slot sums; weights w_j = exp_j / denom[batch, j]
  A-proj: UA/GA [E*R=128, t] = stacked up_A/gate_A @ x.T   (one K=128 chain)
  slot-mask trick: Ut_j = UA * M_j where M_j[e*R+r, t] = (sel_j(t)==e);
          lo_up_j[h,t] = (stacked up_B) @ Ut_j  == up_B[sel_j(t)] @ u_{sel_j(t)}
  h_j = silu(U + lo_up_j) * (G + lo_gate_j); ch_j = c_j * h_j
  mixed = ch_0 + ch_1
  v_j[er,t] = (stacked down_A) @ ch_j  (accumulated over h), masked by M_j
  outT[d,t] = w_down-chain @ mixed + (stacked down_B) @ v_0 + ... @ v_1
"""

import numpy as np

# problem constants (hardcoded; kernel.py must be self-contained)
B, S, D, H, E, R, TOPK = 2, 1024, 2048, 5632, 8, 16, 2
ALPHA = 2.0
T = B * S
NCORES = 8
TC = T // NCORES           # 256 tokens per core
KT = D // 128              # 16 k-tiles over D
HT = H // 128              # 44 h-tiles
DT = D // 128              # 16 d-tiles
ER = E * R                 # 128

MM_MODE = "hyb"            # "f32" | "f32r" | "bf16" | "hyb"
DEBUG_TAPS = False         # add intermediate-tensor outputs for debugging
SKIP_AR = False            # replace AllReduce with local copy (for TimelineSim)

_cache = {}


def _np_sd():
    import ml_dtypes
    return np.dtype(ml_dtypes.bfloat16) if MM_MODE == "bf16" else np.dtype(np.float32)


def _build():
    import concourse.bacc as bacc
    import concourse.bass as bass
    import concourse.mybir as mybir
    import concourse.tile as tile
    from concourse.masks import make_identity

    f32 = mybir.dt.float32
    bf16 = mybir.dt.bfloat16
    SD = bf16 if MM_MODE == "bf16" else f32
    WUG = bf16 if MM_MODE in ("bf16", "hyb") else f32  # up/gate weights + x
    AL = mybir.AluOpType
    AF = mybir.ActivationFunctionType

    def mm(ap):
        # matmul operand dtype override for f32r-path tensors
        if MM_MODE in ("f32r", "hyb"):
            return ap.bitcast(mybir.dt.float32r)
        return ap

    mo = mm  # producer outputs feeding f32r matmuls must also be f32r-typed

    def mug(ap):
        # up/gate-path operands: true bf16 in hyb/bf16, f32r in f32r mode
        if MM_MODE == "f32r":
            return ap.bitcast(mybir.dt.float32r)
        return ap

    nc = bacc.Bacc("TRN2", target_bir_lowering=False, debug=False,
                   num_devices=NCORES)

    # ---- DRAM I/O ----
    d_xT = nc.dram_tensor("xT", [D, TC], SD, kind="ExternalInput").ap()
    if MM_MODE == "bf16":
        d_xTr = nc.dram_tensor("xTr", [D, TC], f32, kind="ExternalInput").ap()
    else:
        d_xTr = d_xT
    d_gw = nc.dram_tensor("gw", [128, KT * E], f32, kind="ExternalInput").ap()
    d_wu = nc.dram_tensor("wu", [HT, 128, KT * 128], WUG, kind="ExternalInput").ap()
    d_wg = nc.dram_tensor("wg", [HT, 128, KT * 128], WUG, kind="ExternalInput").ap()
    d_wd = nc.dram_tensor("wd", [DT, 128, HT * 128], SD, kind="ExternalInput").ap()
    d_A = nc.dram_tensor("Ah", [128, KT * 2 * ER], SD, kind="ExternalInput").ap()
    d_uB = nc.dram_tensor("uB", [HT, 128, 128], SD, kind="ExternalInput").ap()
    d_gB = nc.dram_tensor("gB", [HT, 128, 128], SD, kind="ExternalInput").ap()
    d_dA = nc.dram_tensor("dA", [HT, 128, ER], SD, kind="ExternalInput").ap()
    d_dB = nc.dram_tensor("dB", [128, D], SD, kind="ExternalInput").ap()
    d_eid = nc.dram_tensor("eid", [128, 1], f32, kind="ExternalInput").ap()
    d_i8m = nc.dram_tensor("i8m", [128, E], f32, kind="ExternalInput").ap()
    d_bsr = nc.dram_tensor("bsr", [1, 2], f32, kind="ExternalInput").ap()
    d_bsc = nc.dram_tensor("bsc", [2, 1], f32, kind="ExternalInput").ap()
    d_sel2 = nc.dram_tensor("sel2", [2, 256], f32, kind="ExternalInput").ap()
    d_out = nc.dram_tensor("outT", [D, TC], f32, kind="ExternalOutput").ap()

    with tile.TileContext(nc) as tc:
        import contextlib
        ctx = contextlib.ExitStack()
        with ctx:
            cpool = ctx.enter_context(tc.tile_pool(name="const", bufs=1))
            wpool = ctx.enter_context(tc.tile_pool(name="wstream", bufs=2))
            bpool = ctx.enter_context(tc.tile_pool(name="bstream", bufs=3))
            spool = ctx.enter_context(tc.tile_pool(name="work", bufs=2))
            pspool = ctx.enter_context(
                tc.tile_pool(name="ps", bufs=1, space="PSUM"))
            drpool = ctx.enter_context(
                tc.tile_pool(name="dram", bufs=1, space="DRAM"))

            # ---- constants / resident tiles ----
            xT_sb = cpool.tile([128, KT * TC], SD, name="xT_sb")
            for k in range(KT):
                nc.sync.dma_start(out=mo(xT_sb[:, k * TC:(k + 1) * TC]),
                                  in_=mo(d_xT[k * 128:(k + 1) * 128, :]))
            if MM_MODE == "bf16":
                xTr_sb = cpool.tile([128, KT * TC], f32, name="xTr_sb")
                for k in range(KT):
                    nc.sync.dma_start(out=xTr_sb[:, k * TC:(k + 1) * TC],
                                      in_=d_xTr[k * 128:(k + 1) * 128, :])
            elif MM_MODE in ("f32r", "hyb"):
                xTr_sb = xT_sb.bitcast(f32)   # same bits, f32 view for router
            else:
                xTr_sb = xT_sb
            if MM_MODE == "hyb":
                # bf16 copy of x for the up/gate base GEMMs (gpsimd casts)
                xTb = cpool.tile([128, KT * TC], bf16, name="xTb")
                for k in range(KT):
                    nc.gpsimd.dma_start(out=xTb[:, k * TC:(k + 1) * TC],
                                        in_=d_xT[k * 128:(k + 1) * 128, :])
            else:
                xTb = xT_sb
            A_sb = cpool.tile([128, KT * 2 * ER], SD, name="A_sb")
            nc.sync.dma_start(out=mo(A_sb[:]), in_=mo(d_A[:]))
            dB_sb = cpool.tile([128, D], SD, name="dB_sb")
            nc.sync.dma_start(out=mo(dB_sb[:]), in_=mo(d_dB[:]))
            gw_sb = cpool.tile([128, KT * E], f32, name="gw_sb")
            nc.sync.dma_start(out=gw_sb[:], in_=d_gw[:])
            eid_sb = cpool.tile([128, 1], f32, name="eid_sb")
            nc.sync.dma_start(out=eid_sb[:], in_=d_eid[:])
            i8m_sb = cpool.tile([128, E], f32, name="i8m_sb")
            nc.sync.dma_start(out=i8m_sb[:], in_=d_i8m[:])
            bsr_sb = cpool.tile([1, 2], f32, name="bsr_sb")
            nc.sync.dma_start(out=bsr_sb[:], in_=d_bsr[:])
            bsc_sb = cpool.tile([2, 1], f32, name="bsc_sb")
            nc.sync.dma_start(out=bsc_sb[:], in_=d_bsc[:])
            sel2_sb = cpool.tile([2, 256], f32, name="sel2_sb")
            nc.sync.dma_start(out=sel2_sb[:], in_=d_sel2[:])

            ident = cpool.tile([128, 128], f32, name="ident")
            make_identity(nc, ident)
            ones_row = cpool.tile([1, 128], f32, name="ones_row")
            nc.vector.memset(ones_row, 1.0)
            ones_col = cpool.tile([128, 1], f32, name="ones_col")
            nc.vector.memset(ones_col, 1.0)

            mixed = cpool.tile([128, HT * TC], SD, name="mixed")
            ev_rows = cpool.tile([2, TC], f32, name="ev_rows")
            s_rows = cpool.tile([2, TC], f32, name="s_rows")
            crows = cpool.tile([2, TC], f32, name="crows")
            cb = cpool.tile([128, 2 * TC], SD, name="cb")
            Mj = cpool.tile([128, 2 * TC], SD, name="Mj")
            UA = cpool.tile([128, TC], SD, name="UA")
            GA = cpool.tile([128, TC], SD, name="GA")
            Ut = cpool.tile([128, 2 * TC], SD, name="Ut")
            Gt = cpool.tile([128, 2 * TC], SD, name="Gt")
            vt = cpool.tile([128, 2 * TC], SD, name="vt")

            # ---- phase 1: router ----
            den_parts = cpool.tile([1, 4], f32, name="den_parts")
            for tt in range(2):
                psL = pspool.tile([128, TC], f32, tag="ps_small", name="psL")
                for k in range(KT):
                    nc.tensor.matmul(
                        psL[:, 0:E],
                        xTr_sb[:, k * TC + tt * 128: k * TC + tt * 128 + 128],
                        gw_sb[:, k * E:(k + 1) * E],
                        start=(k == 0), stop=(k == KT - 1))
                L = spool.tile([128, E], f32, tag="L")
                nc.vector.tensor_copy(L[:], psL[:, 0:E])
                mx1 = spool.tile([128, 1], f32, tag="mx1")
                nc.vector.tensor_reduce(mx1[:], L[:], mybir.AxisListType.X, AL.max)
                msk = spool.tile([128, E], f32, tag="msk")
                nc.vector.tensor_scalar(msk[:], L[:], mx1[:], None, AL.is_equal)
                mi = spool.tile([128, E], f32, tag="mi")
                nc.vector.tensor_tensor(mi[:], msk[:], i8m_sb[:], AL.mult)
                svals = spool.tile([128, 2], f32, tag="svals")
                nc.vector.tensor_reduce(svals[:, 0:1], mi[:],
                                        mybir.AxisListType.X, AL.max)
                evals = spool.tile([128, 2], f32, tag="evals")
                nc.scalar.activation(evals[:, 0:1], mx1[:], AF.Exp)
                # mask out slot-0 winner, find second max
                big = spool.tile([128, E], f32, tag="big")
                nc.vector.tensor_scalar(big[:], msk[:], 1e30, None, AL.mult)
                L2 = spool.tile([128, E], f32, tag="L2")
                nc.vector.tensor_tensor(L2[:], L[:], big[:], AL.subtract)
                mx2 = spool.tile([128, 1], f32, tag="mx2")
                nc.vector.tensor_reduce(mx2[:], L2[:], mybir.AxisListType.X, AL.max)
                msk2 = spool.tile([128, E], f32, tag="msk2")
                nc.vector.tensor_scalar(msk2[:], L2[:], mx2[:], None, AL.is_equal)
                mi2 = spool.tile([128, E], f32, tag="mi2")
                nc.vector.tensor_tensor(mi2[:], msk2[:], i8m_sb[:], AL.mult)
                nc.vector.tensor_reduce(svals[:, 1:2], mi2[:],
                                        mybir.AxisListType.X, AL.max)
                nc.scalar.activation(evals[:, 1:2], mx2[:], AF.Exp)
                # per-tile partial denominators: [1,2] = ones.T @ evals
                psd = pspool.tile([1, 2], f32, tag="ps_small", name="psd")
                nc.tensor.matmul(psd[:], ones_col[:], evals[:],
                                 start=True, stop=True)
                nc.vector.tensor_copy(den_parts[:, tt * 2:(tt + 1) * 2], psd[:])
                # transpose evals/svals -> rows
                psT = pspool.tile([2, 128], f32, tag="ps_small", name="psT")
                nc.tensor.transpose(psT[:], evals[:], ident[:])
                nc.vector.tensor_copy(ev_rows[:, tt * 128:(tt + 1) * 128], psT[:])
                psT2 = pspool.tile([2, 128], f32, tag="ps_small", name="psT2")
                nc.tensor.transpose(psT2[:], svals[:], ident[:])
                nc.vector.tensor_copy(s_rows[:, tt * 128:(tt + 1) * 128], psT2[:])

            # combine partials, AllReduce [2,8] (row b = batch, cols 0:2 used)
            drow = cpool.tile([1, 2], f32, name="drow")
            nc.vector.tensor_tensor(drow[:], den_parts[:, 0:2],
                                    den_parts[:, 2:4], AL.add)
            ar_sb = cpool.tile([2, 8], f32, name="ar_sb")
            nc.vector.memset(ar_sb, 0.0)
            psAR = pspool.tile([2, 2], f32, tag="ps_small", name="psAR")
            nc.tensor.matmul(psAR[:], bsr_sb[:], drow[:], start=True, stop=True)
            nc.vector.tensor_copy(ar_sb[:, 0:2], psAR[:])
            ar_in = drpool.tile([2, 8], f32, name="ar_in")
            ar_out = drpool.tile([2, 8], f32, name="ar_out", addr_space="Shared")
            nc.gpsimd.dma_start(out=ar_in[:], in_=ar_sb[:])
            if SKIP_AR:
                nc.gpsimd.dma_start(out=ar_out[:], in_=ar_in[:])
            else:
                nc.gpsimd.collective_compute(
                    "AllReduce", AL.add,
                    replica_groups=[list(range(NCORES))],
                    ins=[ar_in.opt()], outs=[ar_out.opt()])
            den2 = cpool.tile([2, 8], f32, name="den2")
            nc.gpsimd.dma_start(out=den2[:], in_=ar_out[:])
            # select this core's batch row -> [2(slots),1], reciprocal
            psDC = pspool.tile([2, 1], f32, tag="ps_small", name="psDC")
            nc.tensor.matmul(psDC[:], den2[:, 0:2], bsc_sb[:],
                             start=True, stop=True)
            rcp = cpool.tile([2, 1], f32, name="rcp")
            nc.vector.reciprocal(rcp[:], psDC[:])
            # normalized routing weights as rows [2, TC]
            nc.vector.tensor_scalar(crows[:], ev_rows[:], rcp[:], None, AL.mult)

            # broadcast slot rows along partitions via K=2 matmul with a
            # row-selector constant (sel2[:, j*128:(j+1)*128] has row j = 1)
            for j in range(2):
                psB = pspool.tile([128, TC], f32, tag="ps_small", name="psB")
                nc.tensor.matmul(psB[:], sel2_sb[:, j * 128:(j + 1) * 128],
                                 crows[:], start=True, stop=True)
                nc.vector.tensor_copy(cb[:, j * TC:(j + 1) * TC], psB[:])
                psM = pspool.tile([128, TC], f32, tag="ps_small", name="psM")
                nc.tensor.matmul(psM[:], sel2_sb[:, j * 128:(j + 1) * 128],
                                 s_rows[:], start=True, stop=True)
                nc.vector.tensor_scalar(Mj[:, j * TC:(j + 1) * TC], psM[:],
                                        eid_sb[:], None, AL.is_equal)

            # ---- phase 3: stacked A-projections ----
            psUA = pspool.tile([128, TC], f32, tag="psUG", bufs=2, name="psUA")
            for k in range(KT):
                nc.tensor.matmul(psUA[:],
                                 mm(A_sb[:, k * 2 * ER: k * 2 * ER + ER]),
                                 mm(xT_sb[:, k * TC:(k + 1) * TC]),
                                 start=(k == 0), stop=(k == KT - 1))
            nc.vector.tensor_copy(UA[:], psUA[:])
            psGA = pspool.tile([128, TC], f32, tag="psUG", bufs=2, name="psGA")
            for k in range(KT):
                nc.tensor.matmul(psGA[:],
                                 mm(A_sb[:, k * 2 * ER + ER:(k + 1) * 2 * ER]),
                                 mm(xT_sb[:, k * TC:(k + 1) * TC]),
                                 start=(k == 0), stop=(k == KT - 1))
            nc.vector.tensor_copy(GA[:], psGA[:])
            for j in range(2):
                nc.vector.tensor_tensor(mo(Ut[:, j * TC:(j + 1) * TC]), UA[:],
                                        Mj[:, j * TC:(j + 1) * TC], AL.mult)
                nc.vector.tensor_tensor(mo(Gt[:, j * TC:(j + 1) * TC]), GA[:],
                                        Mj[:, j * TC:(j + 1) * TC], AL.mult)

            # ---- phases 2+5+6: h-tile loop ----
            psV = pspool.tile([128, 2 * TC], f32, tag="psV", name="psV")
            KH = KT // 2 * 128          # half of the k columns (1024)
            QH = HT // 4 * 128          # quarter of the h columns (1408)
            wd_pre = {}                 # (di, q) -> prefetched tile
            pend_v = None               # delayed psV matmul (dA_t, ch_pair)

            def load_wd(di, q):
                t = wpool.tile([128, QH], SD, tag="wd", bufs=6, name="wd_t")
                nc.sync.dma_start(
                    out=mo(t[:]), in_=mo(d_wd[di][:, q * QH:(q + 1) * QH]))
                return t

            for i in range(HT):
                if 4 <= i < 10:
                    k6 = i - 4          # prefetch 6 wd quarters mid-loop
                    wd_pre[(k6 // 4, k6 % 4)] = load_wd(k6 // 4, k6 % 4)
                wu_h = []
                wg_h = []
                for hf in range(2):
                    wu_t = wpool.tile([128, KH], WUG, tag="wu", bufs=4)
                    nc.sync.dma_start(
                        out=mug(wu_t[:]),
                        in_=mug(d_wu[i][:, hf * KH:(hf + 1) * KH]))
                    wu_h.append(wu_t)
                    wg_t = wpool.tile([128, KH], WUG, tag="wg", bufs=4)
                    nc.sync.dma_start(
                        out=mug(wg_t[:]),
                        in_=mug(d_wg[i][:, hf * KH:(hf + 1) * KH]))
                    wg_h.append(wg_t)
                uB_t = bpool.tile([128, 128], SD, tag="uB")
                nc.sync.dma_start(out=mo(uB_t[:]), in_=mo(d_uB[i]))
                gB_t = bpool.tile([128, 128], SD, tag="gB")
                nc.sync.dma_start(out=mo(gB_t[:]), in_=mo(d_gB[i]))
                dA_t = bpool.tile([128, ER], SD, tag="dA")
                nc.sync.dma_start(out=mo(dA_t[:]), in_=mo(d_dA[i]))

                psUG = pspool.tile([128, 2 * TC], f32, tag="psUG", bufs=2,
                                   name="psUG")
                for k in range(KT):
                    w = wu_h[k // 8][:, (k % 8) * 128:(k % 8 + 1) * 128]
                    nc.tensor.matmul(psUG[:, 0:TC], mug(w),
                                     mug(xTb[:, k * TC:(k + 1) * TC]),
                                     start=(k == 0), stop=(k == KT - 1))
                for k in range(KT):
                    w = wg_h[k // 8][:, (k % 8) * 128:(k % 8 + 1) * 128]
                    nc.tensor.matmul(psUG[:, TC:2 * TC], mug(w),
                                     mug(xTb[:, k * TC:(k + 1) * TC]),
                                     start=(k == 0), stop=(k == KT - 1))
                if pend_v is not None:
                    pv_dA, pv_ch = pend_v
                    nc.tensor.matmul(psV[:], mm(pv_dA[:]), mm(pv_ch[:]),
                                     start=(i == 1), stop=False,
                                     skip_group_check=True)
                U_sb = spool.tile([128, TC], SD, tag="U_sb")
                nc.scalar.copy(U_sb[:], psUG[:, 0:TC])
                G_sb = spool.tile([128, TC], SD, tag="G_sb")
                nc.scalar.copy(G_sb[:], psUG[:, TC:2 * TC])

                psLO = pspool.tile([128, 4 * TC], f32, tag="psLO", bufs=2,
                                   name="psLO")
                # both slots' c*h in ONE tile so the down_A contraction is a
                # single [128,512] matmul per h-tile (avoids the whole-bank
                # has_written clear from a second start=True in the same bank)
                ch_pair = spool.tile([128, 2 * TC], SD, tag="chp", bufs=3)
                for j in range(2):
                    nc.tensor.matmul(psLO[:, (2 * j) * TC:(2 * j + 1) * TC],
                                     mm(uB_t[:]),
                                     mm(Ut[:, j * TC:(j + 1) * TC]),
                                     start=True, stop=True)
                    nc.tensor.matmul(psLO[:, (2 * j + 1) * TC:(2 * j + 2) * TC],
                                     mm(gB_t[:]),
                                     mm(Gt[:, j * TC:(j + 1) * TC]),
                                     start=True, stop=True)
                    tu = spool.tile([128, TC], SD, tag="tu")
                    nc.vector.tensor_tensor(
                        tu[:], U_sb[:], psLO[:, (2 * j) * TC:(2 * j + 1) * TC],
                        AL.add)
                    su = spool.tile([128, TC], SD, tag="su")
                    nc.scalar.activation(su[:], tu[:], AF.Silu)
                    tg = spool.tile([128, TC], SD, tag="tg")
                    nc.vector.tensor_tensor(
                        tg[:], G_sb[:],
                        psLO[:, (2 * j + 1) * TC:(2 * j + 2) * TC], AL.add)
                    hh = spool.tile([128, TC], SD, tag="hh")
                    nc.vector.tensor_tensor(hh[:], su[:], tg[:], AL.mult)
                    nc.vector.tensor_tensor(mo(ch_pair[:, j * TC:(j + 1) * TC]),
                                            hh[:],
                                            cb[:, j * TC:(j + 1) * TC], AL.mult)
                # psV matmul for tile i-1 is emitted AFTER tile i's base
                # matmuls: keeps the in-order PE queue from stalling on the
                # DVE chain that produces ch_pair (head-of-line blocking)
                nc.vector.tensor_tensor(mo(mixed[:, i * TC:(i + 1) * TC]),
                                        ch_pair[:, 0:TC], ch_pair[:, TC:2 * TC],
                                        AL.add)
                pend_v = (dA_t, ch_pair)

            pv_dA, pv_ch = pend_v
            nc.tensor.matmul(psV[:], mm(pv_dA[:]), mm(pv_ch[:]),
                             start=False, stop=True, skip_group_check=True)
            # masked v
            for j in range(2):
                nc.vector.tensor_tensor(mo(vt[:, j * TC:(j + 1) * TC]),
                                        psV[:, j * TC:(j + 1) * TC],
                                        Mj[:, j * TC:(j + 1) * TC], AL.mult)

            if DEBUG_TAPS:
                for nm, tl in [("crows", crows), ("srows", s_rows),
                               ("cb", cb), ("Mj", Mj), ("UA", UA),
                               ("GA", GA), ("vt", vt),
                               ("mixed0", mixed[:, 0:TC]),
                               ("mixed7", mixed[:, 7 * TC:8 * TC])]:
                    shp = [tl.shape[0], tl.shape[-1]]
                    dbg = nc.dram_tensor(f"dbg_{nm}", shp, f32,
                                         kind="ExternalOutput").ap()
                    nc.sync.dma_start(out=dbg[:], in_=tl[:])

            # ---- phase 7: down GEMM + LoRA-down ----
            for di in range(DT):
                wd_q = [wd_pre.get((di, q)) or load_wd(di, q)
                        for q in range(4)]
                psO = pspool.tile([128, TC], f32, tag="psUG", bufs=2, name="psO")
                for hk in range(HT):
                    w = wd_q[hk // 11][:, (hk % 11) * 128:(hk % 11 + 1) * 128]
                    nc.tensor.matmul(psO[:], mm(w),
                                     mm(mixed[:, hk * TC:(hk + 1) * TC]),
                                     start=(hk == 0), stop=False,
                                     skip_group_check=True)
                nc.tensor.matmul(psO[:], mm(dB_sb[:, di * 128:(di + 1) * 128]),
                                 mm(vt[:, 0:TC]), start=False, stop=False,
                                 skip_group_check=True)
                nc.tensor.matmul(psO[:], mm(dB_sb[:, di * 128:(di + 1) * 128]),
                                 mm(vt[:, TC:2 * TC]), start=False, stop=True,
                                 skip_group_check=True)
                o_sb = spool.tile([128, TC], f32, tag="o_sb")
                nc.scalar.copy(o_sb[:], psO[:])
                nc.sync.dma_start(out=d_out[di * 128:(di + 1) * 128, :],
                                  in_=o_sb[:])

    nc.compile()
    return nc


def _prep_shared(inputs):
    """Host-side layout prep of weight tensors (shared across cores)."""
    import ml_dtypes
    sd = _np_sd()
    sd_ug = (np.dtype(ml_dtypes.bfloat16) if MM_MODE in ("bf16", "hyb")
             else np.dtype(np.float32))
    f32 = np.float32

    def c(a, dt):
        return np.ascontiguousarray(a.astype(dt, copy=False))

    w_up, w_gate, w_down = inputs["w_up"], inputs["w_gate"], inputs["w_down"]
    wu = c(w_up.reshape(HT, 128, KT, 128).transpose(0, 3, 2, 1)
           .reshape(HT, 128, KT * 128), sd_ug)
    wg = c(w_gate.reshape(HT, 128, KT, 128).transpose(0, 3, 2, 1)
           .reshape(HT, 128, KT * 128), sd_ug)
    wd = c(w_down.reshape(DT, 128, HT, 128).transpose(0, 3, 2, 1)
           .reshape(DT, 128, HT * 128), sd)

    A_stack = np.concatenate([
        inputs["up_A"].reshape(ER, D),
        inputs["gate_A"].reshape(ER, D)], axis=0)          # [2*ER, D]
    # Ah[p, k*2ER + m] = A_stack[m, k*128+p]
    Ah = c(A_stack.reshape(2 * ER, KT, 128).transpose(2, 1, 0)
           .reshape(128, KT * 2 * ER), sd)

    up_B_all = (inputs["up_B"].transpose(0, 2, 1).reshape(ER, H)
                * ALPHA).astype(f32)
    gate_B_all = (inputs["gate_B"].transpose(0, 2, 1).reshape(ER, H)
                  * ALPHA).astype(f32)
    uB = c(up_B_all.reshape(ER, HT, 128).transpose(1, 0, 2), sd)
    gB = c(gate_B_all.reshape(ER, HT, 128).transpose(1, 0, 2), sd)

    down_A_all = inputs["down_A"].reshape(ER, H).astype(f32)
    dA = c(down_A_all.T.reshape(HT, 128, ER), sd)
    down_B_all = (inputs["down_B"].transpose(0, 2, 1).reshape(ER, D)
                  * ALPHA).astype(f32)
    dB = c(down_B_all, sd)

    gate_wT = inputs["gate_w"].T.astype(f32)               # [D, E]
    gw = c(gate_wT.reshape(KT, 128, E).transpose(1, 0, 2)
           .reshape(128, KT * E), f32)

    eid = (8.0 - (np.arange(128) // R)).astype(f32).reshape(128, 1)
    i8m = np.tile((8.0 - np.arange(E)).astype(f32), (128, 1))
    sel2 = np.zeros((2, 256), f32)
    sel2[0, 0:128] = 1.0
    sel2[1, 128:256] = 1.0

    return dict(wu=wu, wg=wg, wd=wd, Ah=Ah, uB=uB, gB=gB, dA=dA, dB=dB,
                gw=gw, eid=eid, i8m=i8m, sel2=sel2)


def kernel(**inputs):
    from concourse.bass_utils import run_bass_kernel_spmd

    inputs = {k: np.asarray(v) for k, v in inputs.items()}
    if "nc" not in _cache:
        _cache["nc"] = _build()
    nc = _cache["nc"]

    shared = _prep_shared(inputs)
    sd = _np_sd()
    x = inputs["x"].astype(np.float32)
    xt = x.reshape(T, D)

    in_maps = []
    for cix in range(NCORES):
        xc = xt[cix * TC:(cix + 1) * TC]                   # [TC, D]
        xT = np.ascontiguousarray(xc.T)                    # [D, TC] f32
        b = (cix * TC) // S
        bsr = np.zeros((1, 2), np.float32); bsr[0, b] = 1.0
        bsc = np.zeros((2, 1), np.float32); bsc[b, 0] = 1.0
        m = dict(shared)
        m["xT"] = xT.astype(sd) if MM_MODE == "bf16" else xT
        if MM_MODE == "bf16":
            m["xTr"] = xT
        m["bsr"] = bsr
        m["bsc"] = bsc
        in_maps.append(m)

    res = run_bass_kernel_spmd(nc, in_maps, list(range(NCORES)))
    out = np.empty((T, D), np.float32)
    for cix in range(NCORES):
        out[cix * TC:(cix + 1) * TC, :] = res.results[cix]["outT"].T
    return out.reshape(B, S, D)



# revision 29
# speedup vs baseline: 1.4189x; 1.4189x over previous
"""Trainium2 Bass kernel for nn_MistralMoLoraLayer (MoE-routed LoRA FFN).

Strategy: data-parallel over tokens (8 cores x 256 tokens), base FFN weights
replicated in bf16, all-expert LoRA replicated in bf16. The per-(batch,slot)
softmax over the sequence axis needs global denominators -> tiny [2,8]
AllReduce, hidden behind a W-deep base-GEMM software pipeline.

Per-core phases (tiles are [128 partitions, tokens free]):
  router: exact-f32 logits from bf16 pairs (xb+xr)@(gwb+gwr); top-2
          (value,index) per token; exp; AllReduce of per-batch-slot sums;
          weights w_j = exp_j/denom. Emitted in small stages between warmup
          base tiles; only cb (weights) waits on the AR.
  A-proj: UA/GA [E*R=128, t] = stacked up_A/gate_A @ x.T (bf16, K=D chain)
  slot-mask trick: Ut_j = UA * M_j with M_j[e*R+r,t] = (sel_j(t)==e) so a
          single [128,128] B matmul applies the routed expert's LoRA-B
  h-loop: software-pipelined; base GEMM leads the lora stage by W tiles to
          hide router+AR latency, then double-lora iterations shrink the
          lag to 3 so there is no tail drain.
      base: psUG = [w_up|w_gate] chain @ x  -> U/G ring (bf16, Act copies)
      lora: psLOu/g = up_B/gate_B @ Ut/Gt (double-buffered banks);
            h_j = silu(U+lo_u)*(G+lo_g) with slot0 via Act copies and
            slot1 via direct DVE PSUM reads (engine balance);
            ch_j = c_j*h_j; mixed = ch_0+ch_1; psV += down_A @ ch (lag 2)
  down:   outT = w_down chain @ mixed + down_B @ (v_0+v_1); wd streamed in
          quarters, 5 d-tiles prefetched during the h-loop
"""

import numpy as np

# problem constants (hardcoded; kernel.py must be self-contained)
B, S, D, H, E, R, TOPK = 2, 1024, 2048, 5632, 8, 16, 2
ALPHA = 2.0
T = B * S
NCORES = 8
TC = T // NCORES           # 256 tokens per core
KT = D // 128              # 16 k-tiles over D
HT = H // 128              # 44 h-tiles
DT = D // 128              # 16 d-tiles
ER = E * R                 # 128

W = 12                     # base-GEMM lead (pipeline depth, h-tiles)
NPRE = 5                   # wd d-tiles prefetched during h-loop
DEBUG_TAPS = False
SKIP_AR = False            # replace AllReduce with local copy (TimelineSim)

_cache = {}


def _build():
    import concourse.bacc as bacc
    import concourse.bass as bass
    import concourse.mybir as mybir
    import concourse.tile as tile
    from concourse.masks import make_identity

    f32 = mybir.dt.float32
    bf16 = mybir.dt.bfloat16
    AL = mybir.AluOpType
    AF = mybir.ActivationFunctionType

    nc = bacc.Bacc("TRN2", target_bir_lowering=False, debug=False,
                   num_devices=NCORES)

    # ---- DRAM I/O ----
    d_xTb = nc.dram_tensor("xTb", [128, KT * TC], bf16,
                           kind="ExternalInput").ap()
    d_xr = nc.dram_tensor("xr", [128, KT * TC], bf16,
                          kind="ExternalInput").ap()
    d_gwb = nc.dram_tensor("gwb", [128, KT * E], bf16,
                           kind="ExternalInput").ap()
    d_gwr = nc.dram_tensor("gwr", [128, KT * E], bf16,
                           kind="ExternalInput").ap()
    d_wug = nc.dram_tensor("wug", [HT, 128, 2 * KT * 128], bf16,
                           kind="ExternalInput").ap()
    d_wd = nc.dram_tensor("wd", [DT, 128, HT * 128], bf16,
                          kind="ExternalInput").ap()
    d_A = nc.dram_tensor("Ah", [128, KT * 2 * ER], bf16,
                         kind="ExternalInput").ap()
    d_lora = nc.dram_tensor("lorah", [HT, 128, 3 * 128], bf16,
                            kind="ExternalInput").ap()
    d_dB = nc.dram_tensor("dB", [128, D], bf16, kind="ExternalInput").ap()
    d_eid = nc.dram_tensor("eid", [128, 1], f32, kind="ExternalInput").ap()
    d_i8m = nc.dram_tensor("i8m", [128, E], f32, kind="ExternalInput").ap()
    d_bsr = nc.dram_tensor("bsr", [1, 2], f32, kind="ExternalInput").ap()
    d_bsc = nc.dram_tensor("bsc", [2, 1], f32, kind="ExternalInput").ap()
    d_sel2 = nc.dram_tensor("sel2", [2, 256], f32, kind="ExternalInput").ap()
    d_out = nc.dram_tensor("outT", [D, TC], f32, kind="ExternalOutput").ap()

    with tile.TileContext(nc) as tc:
        import contextlib
        ctx = contextlib.ExitStack()
        with ctx:
            cpool = ctx.enter_context(tc.tile_pool(name="const", bufs=1))
            wpool = ctx.enter_context(tc.tile_pool(name="wstream", bufs=2))
            spool = ctx.enter_context(tc.tile_pool(name="work", bufs=2))
            pspool = ctx.enter_context(
                tc.tile_pool(name="ps", bufs=1, space="PSUM"))
            drpool = ctx.enter_context(
                tc.tile_pool(name="dram", bufs=1, space="DRAM"))

            # ---- prologue DMAs: base-GEMM inputs first (PE starts on
            # base(0) immediately; router waits for xT behind it) ----
            xTb_sb = cpool.tile([128, KT * TC], bf16, name="xTb_sb")
            XQ = KT * TC // 4

            def load_xtb(q):
                nc.sync.dma_start(out=xTb_sb[:, q * XQ:(q + 1) * XQ],
                                  in_=d_xTb[:, q * XQ:(q + 1) * XQ])
            load_xtb(0)

            # streamed weights: one DMA per h-tile
            def load_wug(i):
                t = wpool.tile([128, 2 * KT * 128], bf16, tag="wug", bufs=4,
                               name="wug_t")
                nc.sync.dma_start(out=t[:], in_=d_wug[i])
                return t

            def load_lora(i):
                t = wpool.tile([128, 3 * 128], bf16, tag="lora", bufs=W + 6,
                               name="lora_t")
                nc.sync.dma_start(out=t[:], in_=d_lora[i])
                return t

            WDQ = HT // 4 * 128        # quarter of wd's free columns (1408)

            def load_wd_q(di, q):
                t = wpool.tile([128, WDQ], bf16, tag="wd", bufs=26,
                               name="wd_t")
                nc.sync.dma_start(out=t[:], in_=d_wd[di][:, q * WDQ:
                                                         (q + 1) * WDQ])
                return t

            # tile 0's weights in pieces so base(0) starts ~2us in
            wug0 = wpool.tile([128, 2 * KT * 128], bf16, tag="wug", bufs=4,
                              name="wug0")
            UGH = KT * 128
            nc.sync.dma_start(out=wug0[:, 0:UGH // 2],
                              in_=d_wug[0][:, 0:UGH // 2])
            load_xtb(1)
            nc.sync.dma_start(out=wug0[:, UGH // 2:UGH],
                              in_=d_wug[0][:, UGH // 2:UGH])
            load_xtb(2)
            load_xtb(3)
            nc.sync.dma_start(out=wug0[:, UGH:2 * UGH],
                              in_=d_wug[0][:, UGH:2 * UGH])
            wug_t = {0: wug0}
            wug_t[1] = load_wug(1)
            wug_t[2] = load_wug(2)
            gwb_sb = cpool.tile([128, KT * E], bf16, name="gwb_sb")
            nc.sync.dma_start(out=gwb_sb[:], in_=d_gwb[:])
            gwr_sb = cpool.tile([128, KT * E], bf16, name="gwr_sb")
            nc.sync.dma_start(out=gwr_sb[:], in_=d_gwr[:])
            xr_sb = cpool.tile([128, KT * TC], bf16, name="xr_sb")
            nc.sync.dma_start(out=xr_sb[:], in_=d_xr[:])
            wug_t[3] = load_wug(3)
            A_sb = cpool.tile([128, KT * 2 * ER], bf16, name="A_sb")
            nc.sync.dma_start(out=A_sb[:], in_=d_A[:])
            eid_sb = cpool.tile([128, 1], f32, name="eid_sb")
            nc.sync.dma_start(out=eid_sb[:], in_=d_eid[:])
            i8m_sb = cpool.tile([128, E], f32, name="i8m_sb")
            nc.sync.dma_start(out=i8m_sb[:], in_=d_i8m[:])
            bsr_sb = cpool.tile([1, 2], f32, name="bsr_sb")
            nc.sync.dma_start(out=bsr_sb[:], in_=d_bsr[:])
            bsc_sb = cpool.tile([2, 1], f32, name="bsc_sb")
            nc.sync.dma_start(out=bsc_sb[:], in_=d_bsc[:])
            sel2_sb = cpool.tile([2, 256], f32, name="sel2_sb")
            nc.sync.dma_start(out=sel2_sb[:], in_=d_sel2[:])
            lora_t = {i: load_lora(i) for i in range(4)}
            dB_sb = cpool.tile([128, D], bf16, name="dB_sb")

            ident = cpool.tile([128, 128], f32, name="ident")
            make_identity(nc, ident)
            ones_col = cpool.tile([128, 1], f32, name="ones_col")
            nc.vector.memset(ones_col, 1.0)

            mixed = cpool.tile([128, HT * TC], bf16, name="mixed")
            ev_rows = cpool.tile([2, TC], f32, name="ev_rows")
            s_rows = cpool.tile([2, TC], f32, name="s_rows")
            crows = cpool.tile([2, TC], f32, name="crows")
            cb = cpool.tile([128, 2 * TC], bf16, name="cb")
            Mj = cpool.tile([128, 2 * TC], bf16, name="Mj")
            UA = cpool.tile([128, TC], bf16, name="UA")
            GA = cpool.tile([128, TC], bf16, name="GA")
            Ut = cpool.tile([128, 2 * TC], bf16, name="Ut")
            Gt = cpool.tile([128, 2 * TC], bf16, name="Gt")
            vt = cpool.tile([128, 2 * TC], bf16, name="vt")
            vts = cpool.tile([128, TC], bf16, name="vts")

            # ---- phase 1: router (f32), emitted in small stages between
            # warmup base tiles so no PE op waits long on DVE/collective.
            # Only cb (routing weights) depends on the AllReduce; masks Mj
            # and Ut/Gt do not, so the AR only gates the tail of lora(0).
            den_parts = cpool.tile([1, 4], f32, name="den_parts")
            tk = {}

            def emit_logits(tt):
                psL = pspool.tile([128, TC], f32, tag="ps_small", name="psL")
                # exact f32 logits from bf16 pairs: (xb+xr) @ (gb+gr)
                terms = []
                for k in range(KT):
                    c0 = k * TC + tt * 128
                    for xs in (xTb_sb, xr_sb):
                        for gs in (gwb_sb, gwr_sb):
                            terms.append((xs[:, c0:c0 + 128],
                                          gs[:, k * E:(k + 1) * E]))
                for n, (xs, gs) in enumerate(terms):
                    nc.tensor.matmul(psL[:, 0:E], xs, gs,
                                     start=(n == 0),
                                     stop=(n == len(terms) - 1))
                L = spool.tile([128, E], f32, tag="L", name="L")
                nc.vector.tensor_copy(L[:], psL[:, 0:E])
                mx1 = spool.tile([128, 1], f32, tag="mx1", name="mx1")
                nc.vector.tensor_reduce(mx1[:], L[:], mybir.AxisListType.X,
                                        AL.max)
                msk = spool.tile([128, E], f32, tag="msk", name="msk")
                nc.vector.tensor_scalar(msk[:], L[:], mx1[:], None,
                                        AL.is_equal)
                mi = spool.tile([128, E], f32, tag="mi", name="mi")
                nc.vector.tensor_tensor(mi[:], msk[:], i8m_sb[:], AL.mult)
                svals = spool.tile([128, 2], f32, tag="svals", name="svals")
                nc.vector.tensor_reduce(svals[:, 0:1], mi[:],
                                        mybir.AxisListType.X, AL.max)
                evals = spool.tile([128, 2], f32, tag="evals", name="evals")
                nc.scalar.activation(evals[:, 0:1], mx1[:], AF.Exp)
                # mask out slot-0 winner, find second max
                big = spool.tile([128, E], f32, tag="big", name="big")
                nc.vector.tensor_scalar(big[:], msk[:], 1e30, None, AL.mult)
                L2 = spool.tile([128, E], f32, tag="L2", name="L2")
                nc.vector.tensor_tensor(L2[:], L[:], big[:], AL.subtract)
                mx2 = spool.tile([128, 1], f32, tag="mx2", name="mx2")
                nc.vector.tensor_reduce(mx2[:], L2[:], mybir.AxisListType.X,
                                        AL.max)
                msk2 = spool.tile([128, E], f32, tag="msk2", name="msk2")
                nc.vector.tensor_scalar(msk2[:], L2[:], mx2[:], None,
                                        AL.is_equal)
                mi2 = spool.tile([128, E], f32, tag="mi2", name="mi2")
                nc.vector.tensor_tensor(mi2[:], msk2[:], i8m_sb[:], AL.mult)
                nc.vector.tensor_reduce(svals[:, 1:2], mi2[:],
                                        mybir.AxisListType.X, AL.max)
                nc.scalar.activation(evals[:, 1:2], mx2[:], AF.Exp)
                tk[tt] = (evals, svals)

            def emit_topk(tt):
                evals, svals = tk[tt]
                # per-tile partial denominators: [1,2] = ones.T @ evals
                psd = pspool.tile([1, 2], f32, tag="ps_small", name="psd")
                nc.tensor.matmul(psd[:], ones_col[:], evals[:],
                                 start=True, stop=True)
                nc.vector.tensor_copy(den_parts[:, tt * 2:(tt + 1) * 2],
                                      psd[:])
                # transpose evals/svals -> rows
                psT = pspool.tile([2, 128], f32, tag="ps_small", name="psT")
                nc.tensor.transpose(psT[:], evals[:], ident[:])
                nc.vector.tensor_copy(ev_rows[:, tt * 128:(tt + 1) * 128],
                                      psT[:])
                psT2 = pspool.tile([2, 128], f32, tag="ps_small", name="psT2")
                nc.tensor.transpose(psT2[:], svals[:], ident[:])
                nc.vector.tensor_copy(s_rows[:, tt * 128:(tt + 1) * 128],
                                      psT2[:])

            def emit_ar():
                # combine partials, AllReduce [2,8] (row b = batch)
                drow = cpool.tile([1, 2], f32, name="drow")
                nc.vector.tensor_tensor(drow[:], den_parts[:, 0:2],
                                        den_parts[:, 2:4], AL.add)
                ar_sb = cpool.tile([2, 8], f32, name="ar_sb")
                nc.vector.memset(ar_sb, 0.0)
                psAR = pspool.tile([2, 2], f32, tag="ps_small", name="psAR")
                nc.tensor.matmul(psAR[:], bsr_sb[:], drow[:], start=True,
                                 stop=True)
                nc.vector.tensor_copy(ar_sb[:, 0:2], psAR[:])
                ar_in = drpool.tile([2, 8], f32, name="ar_in")
                ar_out = drpool.tile([2, 8], f32, name="ar_out",
                                     addr_space="Shared")
                nc.gpsimd.dma_start(out=ar_in[:], in_=ar_sb[:])
                if SKIP_AR:
                    nc.gpsimd.dma_start(out=ar_out[:], in_=ar_in[:])
                else:
                    nc.gpsimd.collective_compute(
                        "AllReduce", AL.add,
                        replica_groups=[list(range(NCORES))],
                        ins=[ar_in.opt()], outs=[ar_out.opt()])
                den2 = cpool.tile([2, 8], f32, name="den2")
                nc.gpsimd.dma_start(out=den2[:], in_=ar_out[:])
                tk["den2"] = den2

            def emit_denrecv():
                # select this core's batch row -> [2(slots),1], reciprocal
                psDC = pspool.tile([2, 1], f32, tag="ps_small", name="psDC")
                nc.tensor.matmul(psDC[:], tk["den2"][:, 0:2], bsc_sb[:],
                                 start=True, stop=True)
                rcp = cpool.tile([2, 1], f32, name="rcp")
                nc.vector.reciprocal(rcp[:], psDC[:])
                # normalized routing weights as rows [2, TC]
                nc.vector.tensor_scalar(crows[:], ev_rows[:], rcp[:], None,
                                        AL.mult)

            def emit_mj():
                # masks from top-k indices (no AR dependency), then Ut/Gt
                for j in range(2):
                    psM = pspool.tile([128, TC], f32, tag="ps_small",
                                      name="psM")
                    nc.tensor.matmul(psM[:],
                                     sel2_sb[:, j * 128:(j + 1) * 128],
                                     s_rows[:], start=True, stop=True)
                    nc.vector.tensor_scalar(Mj[:, j * TC:(j + 1) * TC],
                                            psM[:], eid_sb[:], None,
                                            AL.is_equal)
                for j in range(2):
                    nc.vector.tensor_tensor(Ut[:, j * TC:(j + 1) * TC],
                                            UA[:],
                                            Mj[:, j * TC:(j + 1) * TC],
                                            AL.mult)
                    nc.vector.tensor_tensor(Gt[:, j * TC:(j + 1) * TC],
                                            GA[:],
                                            Mj[:, j * TC:(j + 1) * TC],
                                            AL.mult)

            def emit_cb():
                # routing weights broadcast along partitions (AR-dependent)
                for j in range(2):
                    psB = pspool.tile([128, TC], f32, tag="ps_small",
                                      name="psB")
                    nc.tensor.matmul(psB[:],
                                     sel2_sb[:, j * 128:(j + 1) * 128],
                                     crows[:], start=True, stop=True)
                    nc.vector.tensor_copy(cb[:, j * TC:(j + 1) * TC], psB[:])

            # ---- phase 3: stacked A-projections (bf16) ----
            def emit_aproj():
                psUA = pspool.tile([128, TC], f32, tag="psUG", bufs=2,
                                   name="psUA")
                for k in range(KT):
                    nc.tensor.matmul(psUA[:],
                                     A_sb[:, k * 2 * ER: k * 2 * ER + ER],
                                     xTb_sb[:, k * TC:(k + 1) * TC],
                                     start=(k == 0), stop=(k == KT - 1))
                nc.scalar.copy(UA[:], psUA[:])
                psGA = pspool.tile([128, TC], f32, tag="psUG", bufs=2,
                                   name="psGA")
                for k in range(KT):
                    nc.tensor.matmul(
                        psGA[:],
                        A_sb[:, k * 2 * ER + ER:(k + 1) * 2 * ER],
                        xTb_sb[:, k * TC:(k + 1) * TC],
                        start=(k == 0), stop=(k == KT - 1))
                nc.scalar.copy(GA[:], psGA[:])

            # ---- h-loop: software pipeline, base leads lora by W tiles,
            # then double-lora iterations shrink the lag to 3 (no tail drain)
            psV = pspool.tile([128, 2 * TC], f32, tag="psV", name="psV")
            U_ring = {}
            G_ring = {}
            pend = []                  # lora indices awaiting psV emission
            pend_t = {}                # i -> (dA slice, ch_pair)
            wd_pre = {}                # (di, q) -> tile
            wd_sched = []              # (iteration, di, q) prefetch slots
            for n in range(NPRE * 4):
                wd_sched.append((1 + n if n < 8 else 9 + 2 * (n - 8),
                                 n // 4, n % 4))
            wd_ptr = 0

            def base_tile(j):
                wt = wug_t.pop(j)
                psUG = pspool.tile([128, 2 * TC], f32, tag="psUG", bufs=2,
                                   name="psUG")
                for k in range(KT):
                    nc.tensor.matmul(psUG[:, 0:TC],
                                     wt[:, k * 128:(k + 1) * 128],
                                     xTb_sb[:, k * TC:(k + 1) * TC],
                                     start=(k == 0), stop=(k == KT - 1))
                for k in range(KT):
                    nc.tensor.matmul(psUG[:, TC:2 * TC],
                                     wt[:, (KT + k) * 128:(KT + k + 1) * 128],
                                     xTb_sb[:, k * TC:(k + 1) * TC],
                                     start=(k == 0), stop=(k == KT - 1))
                U_sb = spool.tile([128, TC], bf16, tag="U_sb", bufs=W + 2,
                                  name="U_sb")
                nc.scalar.copy(U_sb[:], psUG[:, 0:TC])
                G_sb = spool.tile([128, TC], bf16, tag="G_sb", bufs=W + 2,
                                  name="G_sb")
                nc.scalar.copy(G_sb[:], psUG[:, TC:2 * TC])
                U_ring[j] = U_sb
                G_ring[j] = G_sb

            def flush_psv(upto):
                # emit psV contractions for pending loras <= upto
                while pend and pend[0] <= upto:
                    l = pend.pop(0)
                    pv_dA, pv_ch = pend_t.pop(l)
                    nc.tensor.matmul(psV[:], pv_dA, pv_ch[:],
                                     start=(l == 0), stop=(l == HT - 1),
                                     skip_group_check=True)

            def lora_tile(i):
                lt = lora_t[i]
                psLOu = pspool.tile([128, 2 * TC], f32, tag="psLOu", bufs=2,
                                    name="psLOu")
                psLOg = pspool.tile([128, 2 * TC], f32, tag="psLOg", bufs=2,
                                    name="psLOg")
                nc.tensor.matmul(psLOu[:], lt[:, 0:128], Ut[:],
                                 start=True, stop=True)
                nc.tensor.matmul(psLOg[:], lt[:, 128:256], Gt[:],
                                 start=True, stop=True)
                ch_pair = spool.tile([128, 2 * TC], bf16, tag="chp", bufs=6,
                                     name="ch_pair")
                U_sb = U_ring.pop(i)
                G_sb = G_ring.pop(i)
                # slot 0: Activation copies PSUM->SBUF, DVE adds in bf16
                lu = spool.tile([128, TC], bf16, tag="lu", name="lu")
                nc.scalar.copy(lu[:], psLOu[:, 0:TC])
                lg = spool.tile([128, TC], bf16, tag="lg", name="lg")
                nc.scalar.copy(lg[:], psLOg[:, 0:TC])
                tu0 = spool.tile([128, TC], bf16, tag="tu0", name="tu0")
                nc.vector.tensor_tensor(tu0[:], U_sb[:], lu[:], AL.add)
                tg0 = spool.tile([128, TC], bf16, tag="tg0", name="tg0")
                nc.vector.tensor_tensor(tg0[:], G_sb[:], lg[:], AL.add)
                # slot 1: DVE reads PSUM directly (keeps Activation light)
                tu1 = spool.tile([128, TC], bf16, tag="tu1", name="tu1")
                nc.vector.tensor_tensor(tu1[:], U_sb[:], psLOu[:, TC:2 * TC],
                                        AL.add)
                tg1 = spool.tile([128, TC], bf16, tag="tg1", name="tg1")
                nc.vector.tensor_tensor(tg1[:], G_sb[:], psLOg[:, TC:2 * TC],
                                        AL.add)
                for jj, (tu, tg) in enumerate(((tu0, tg0), (tu1, tg1))):
                    su = spool.tile([128, TC], bf16, tag="su", name="su")
                    nc.scalar.activation(su[:], tu[:], AF.Silu)
                    hh = spool.tile([128, TC], bf16, tag="hh", name="hh")
                    nc.vector.tensor_tensor(hh[:], su[:], tg[:], AL.mult)
                    nc.vector.tensor_tensor(ch_pair[:, jj * TC:(jj + 1) * TC],
                                            hh[:],
                                            cb[:, jj * TC:(jj + 1) * TC],
                                            AL.mult)
                nc.vector.tensor_tensor(mixed[:, i * TC:(i + 1) * TC],
                                        ch_pair[:, 0:TC],
                                        ch_pair[:, TC:2 * TC], AL.add)
                pend.append(i)
                pend_t[i] = (lt[:, 256:384], ch_pair)

            # warmup: W base tiles ahead; router stages interleaved so no
            # PE-queue op ever waits long on a DVE/collective dependency
            for wi in range(W):
                if wi + 4 < HT and wi < W - 1:
                    wug_t[wi + 4] = load_wug(wi + 4)
                base_tile(wi)
                if wi == 2:
                    emit_logits(0)
                elif wi == 3:
                    emit_logits(1)
                elif wi == 4:
                    emit_aproj()
                elif wi == 5:
                    emit_topk(0)
                elif wi == 6:
                    emit_topk(1)
                elif wi == 7:
                    emit_ar()
                elif wi == 8:
                    emit_mj()
                elif wi == W - 2:
                    emit_denrecv()
                elif wi == W - 1:
                    emit_cb()

            li = 0                     # lora cursor
            bi = W                     # base cursor
            nl = 4                     # next lora to load
            it = 0                     # iteration counter (for wd prefetch)
            while li < HT:
                if bi < HT:
                    if bi + 1 < HT and bi + 1 not in wug_t:
                        wug_t[bi + 1] = load_wug(bi + 1)
                    base_tile(bi)
                    bi += 1
                while nl < HT and nl < li + 6:
                    lora_t[nl] = load_lora(nl)
                    nl += 1
                if it == 0:
                    nc.sync.dma_start(out=dB_sb[:], in_=d_dB[:])
                while wd_ptr < len(wd_sched) and wd_sched[wd_ptr][0] <= it:
                    _, pdi, pq = wd_sched[wd_ptr]
                    wd_pre[(pdi, pq)] = load_wd_q(pdi, pq)
                    wd_ptr += 1
                flush_psv(li - 2)
                lora_tile(li)
                li += 1
                if bi - li > 3 and li < HT:
                    flush_psv(li - 2)
                    lora_tile(li)
                    li += 1
                it += 1

            flush_psv(HT - 1)
            # masked v, then fold both slots (down_B is linear)
            nc.vector.tensor_tensor(vt[:], psV[:], Mj[:], AL.mult)
            nc.vector.tensor_tensor(vts[:], vt[:, 0:TC], vt[:, TC:2 * TC],
                                    AL.add)

            # ---- down GEMM + LoRA-down, wd streamed with 2-tile lead ----
            def wd_quarter(di, q):
                t = wd_pre.pop((di, q), None)
                return t if t is not None else load_wd_q(di, q)

            wd_cur = {(di, q): wd_quarter(di, q)
                      for di in range(NPRE + 2) for q in range(4)
                      if di < DT}
            for di in range(DT):
                if di + 2 < DT and (di + 2, 0) not in wd_cur:
                    for q in range(4):
                        wd_cur[(di + 2, q)] = wd_quarter(di + 2, q)
                wd_h = [wd_cur.pop((di, q)) for q in range(4)]
                psO = pspool.tile([128, TC], f32, tag="psUG", bufs=2,
                                  name="psO")
                for hk in range(HT):
                    w = wd_h[hk // 11][:, (hk % 11) * 128:(hk % 11 + 1) * 128]
                    nc.tensor.matmul(psO[:], w,
                                     mixed[:, hk * TC:(hk + 1) * TC],
                                     start=(hk == 0), stop=False,
                                     skip_group_check=True)
                nc.tensor.matmul(psO[:], dB_sb[:, di * 128:(di + 1) * 128],
                                 vts[:], start=False, stop=True,
                                 skip_group_check=True)
                o_sb = spool.tile([128, TC], f32, tag="o_sb", name="o_sb")
                nc.scalar.copy(o_sb[:], psO[:])
                nc.sync.dma_start(out=d_out[di * 128:(di + 1) * 128, :],
                                  in_=o_sb[:])

    nc.compile()
    return nc


def _prep_shared(inputs):
    """Host-side layout prep of weight tensors (shared across cores)."""
    import ml_dtypes
    bf16 = np.dtype(ml_dtypes.bfloat16)
    f32 = np.float32

    def c(a, dt):
        return np.ascontiguousarray(a.astype(dt, copy=False))

    w_up, w_gate, w_down = inputs["w_up"], inputs["w_gate"], inputs["w_down"]
    # wu[i][kp, k*128+h] = w_up[i*128+h, k*128+kp] (lhsT per k-tile)
    wu = (w_up.reshape(HT, 128, KT, 128).transpose(0, 3, 2, 1)
          .reshape(HT, 128, KT * 128))
    wg = (w_gate.reshape(HT, 128, KT, 128).transpose(0, 3, 2, 1)
          .reshape(HT, 128, KT * 128))
    wug = c(np.concatenate([wu, wg], axis=2), bf16)
    wd = c(w_down.reshape(DT, 128, HT, 128).transpose(0, 3, 2, 1)
           .reshape(DT, 128, HT * 128), bf16)

    A_stack = np.concatenate([
        inputs["up_A"].reshape(ER, D),
        inputs["gate_A"].reshape(ER, D)], axis=0)          # [2*ER, D]
    # Ah[p, k*2ER + m] = A_stack[m, k*128+p]
    Ah = c(A_stack.reshape(2 * ER, KT, 128).transpose(2, 1, 0)
           .reshape(128, KT * 2 * ER), bf16)

    up_B_all = (inputs["up_B"].transpose(0, 2, 1).reshape(ER, H)
                * ALPHA).astype(f32)
    gate_B_all = (inputs["gate_B"].transpose(0, 2, 1).reshape(ER, H)
                  * ALPHA).astype(f32)
    uB = up_B_all.reshape(ER, HT, 128).transpose(1, 0, 2)   # [HT, er, h]
    gB = gate_B_all.reshape(ER, HT, 128).transpose(1, 0, 2)
    down_A_all = inputs["down_A"].reshape(ER, H).astype(f32)
    dA = down_A_all.T.reshape(HT, 128, ER)                  # [HT, h, er]
    lorah = c(np.concatenate([uB, gB, dA], axis=2), bf16)   # [HT, 128, 384]

    down_B_all = (inputs["down_B"].transpose(0, 2, 1).reshape(ER, D)
                  * ALPHA).astype(f32)
    dB = c(down_B_all, bf16)

    gate_wT = inputs["gate_w"].T.astype(f32)               # [D, E]
    gw = np.ascontiguousarray(
        gate_wT.reshape(KT, 128, E).transpose(1, 0, 2)
        .reshape(128, KT * E)).astype(f32)
    gwb = gw.astype(bf16)
    gwr = (gw - gwb.astype(f32)).astype(bf16)

    eid = (8.0 - (np.arange(128) // R)).astype(f32).reshape(128, 1)
    i8m = np.tile((8.0 - np.arange(E)).astype(f32), (128, 1))
    sel2 = np.zeros((2, 256), f32)
    sel2[0, 0:128] = 1.0
    sel2[1, 128:256] = 1.0

    return dict(wug=wug, wd=wd, Ah=Ah, lorah=lorah, dB=dB,
                gwb=gwb, gwr=gwr, eid=eid, i8m=i8m, sel2=sel2)


def kernel(**inputs):
    from concourse.bass_utils import run_bass_kernel_spmd
    import ml_dtypes
    bf16 = np.dtype(ml_dtypes.bfloat16)

    inputs = {k: np.asarray(v) for k, v in inputs.items()}
    if "nc" not in _cache:
        _cache["nc"] = _build()
    nc = _cache["nc"]

    shared = _prep_shared(inputs)
    x = inputs["x"].astype(np.float32)
    xt = x.reshape(T, D)

    in_maps = []
    for cix in range(NCORES):
        xc = xt[cix * TC:(cix + 1) * TC]                   # [TC, D]
        # xTh[p, k*TC+t] = xc[t, k*128+p]
        xTh = np.ascontiguousarray(
            xc.T.reshape(KT, 128, TC).transpose(1, 0, 2)
            .reshape(128, KT * TC))
        b = (cix * TC) // S
        bsr = np.zeros((1, 2), np.float32); bsr[0, b] = 1.0
        bsc = np.zeros((2, 1), np.float32); bsc[b, 0] = 1.0
        xb = xTh.astype(bf16)
        xr = (xTh - xb.astype(np.float32)).astype(bf16)
        m = dict(shared)
        m["xTb"] = np.ascontiguousarray(xb)
        m["xr"] = np.ascontiguousarray(xr)
        m["bsr"] = bsr
        m["bsc"] = bsc
        in_maps.append(m)

    res = run_bass_kernel_spmd(nc, in_maps, list(range(NCORES)))
    out = np.empty((T, D), np.float32)
    for cix in range(NCORES):
        out[cix * TC:(cix + 1) * TC, :] = res.results[cix]["outT"].T
    return out.reshape(B, S, D)
